# revision 11
# baseline (speedup 1.0000x reference)
"""MoE top-2 routing kernel for Trainium2, 8-core data-parallel.

Problem: x [524288, 128] f32; gate Linear(128->8); 8 experts Linear(128->128).
  g = softmax(x @ gate_W.T + gate_b); top-2 mask; out = sum_e (g*mask)_e * (x @ W_e.T) + g @ b

The axon tunnel moves ~35-45 MB/s aggregate (shared between directions) and
the host has a single CPU, so wall time = bytes shipped + the host work that
cannot hide under transfers. Device compute is ~0.3 s. This version:
  - x goes up as int8 with per-token scale (64MB instead of 256MB).
  - the gate path (logits/softmax/top-2) runs on the host in f32 (tiny BLAS),
    eliminating the top-2 flips low-precision gating would cause. The device
    receives one packed 20-byte row per token: g[8] bf16, axs=amax_x/127
    bf16, top-2 indices u8[2] (10MB, one upload, one DMA per group); it
    rebuilds the mask, folds the scales, and computes the bias term g @ b on
    the PE (host BLAS is ~2 GFLOP/s, so g @ b there would cost 0.9 s).
  - output returns as int8 + per-token f32 scale packed in one row of 132
    bytes, split into 4 pieces per core: 32 concurrent fetch streams, each
    piece dequantized while the others stream.
  - the jax/shard_map executable is built once and cached; outputs are not
    donated (kernel writes every element); weight/bias consts live on device
    across calls.

Device per core (65536 tokens, 4 pieces x 8 groups of 16 tiles x 128 tokens),
token index = ((piece*8 + group)*128 + partition)*16 + tile so every DMA is
one contiguous strip per partition:
  per group: one ACT upcast of all 16 int8 tiles to bf16, rebuild top-2 mask
    from indices, gmk = g*mask*axs (f32), transpose g into gT for the PE
    bias matmuls; PE transposes batched 8-at-a-time into one PSUM bank with
    a single ACT copy out.
  per tile: 2 bf16 matmuls (all 8 experts), PE bias matmul (gT slice @ b4,
    tile_position by quadrant), DVE broadcast-mult by gmk + ACT copy of the
    bias channel -> sc[9,128], DVE reduce over the 9 channels -> s1 f32.
  per group: abs-max per token, scale = amax/126.5, reciprocal, one DVE
    round-to-nearest quantize to int8, DMA out int8+scale rows.
"""

import sys

if "/opt/trn_rl_repo" not in sys.path:
    sys.path.insert(0, "/opt/trn_rl_repo")

import hashlib
import threading
from concurrent.futures import ThreadPoolExecutor
from contextlib import ExitStack

import ml_dtypes
import numpy as np

import concourse.bass as bass
import concourse.tile as tile
from concourse import bacc
from concourse import mybir

F32 = mybir.dt.float32
BF16 = mybir.dt.bfloat16
I8 = mybir.dt.int8
U8 = mybir.dt.uint8
AF = mybir.ActivationFunctionType
OP = mybir.AluOpType
AX = mybir.AxisListType

N_TOKENS = 524288
D = 128
E = 8
N_CORES = 8
P = 128
G = 32  # tiles per group
SPLITS = 4  # output pieces per core
ROW = D + 4  # oq row: 128 int8 + 4 bytes f32 scale
GROW = 20  # gate row: 8 bf16 g + bf16 axs + 2 u8 idx
QMAX = 126.5  # quant headroom so rounding/reciprocal error cannot wrap int8


def build_nc(shard_tokens: int) -> bass.Bass:
    ntiles = shard_tokens // P
    assert ntiles % (G * SPLITS) == 0
    outer = ntiles // G // SPLITS  # groups per piece
    piece = shard_tokens // SPLITS
    nh = G // 4

    nc = bacc.Bacc()
    xq = nc.dram_tensor("xq", [shard_tokens, D], I8, kind="ExternalInput")
    gaxi = nc.dram_tensor("gaxi", [shard_tokens, GROW], U8, kind="ExternalInput")
    wt = nc.dram_tensor("wt", [D, E * D], BF16, kind="ExternalInput")
    identb = nc.dram_tensor("identb", [P, P], BF16, kind="ExternalInput")
    b_bf = nc.dram_tensor("b_bf", [E, D], BF16, kind="ExternalInput")
    oqcs = [
        nc.dram_tensor(f"oqc{s}", [piece, ROW], I8, kind="ExternalOutput")
        for s in range(SPLITS)
    ]

    x_v = xq.rearrange("(s n p a) d -> s n p a d", s=SPLITS, p=P, a=G)
    ga_v = gaxi.rearrange("(s n p a) c -> s n p a c", s=SPLITS, p=P, a=G)
    oq_vs = [t.rearrange("(n p a) c -> n p a c", p=P, a=G) for t in oqcs]

    with ExitStack() as ctx:
        tc = ctx.enter_context(tile.TileContext(nc))
        consts = ctx.enter_context(tc.tile_pool(name="consts", bufs=1))
        io_pool = ctx.enter_context(tc.tile_pool(name="io", bufs=2))
        xt_pool = ctx.enter_context(tc.tile_pool(name="xts", bufs=2))
        work = ctx.enter_context(tc.tile_pool(name="work", bufs=2))
        gates = ctx.enter_context(tc.tile_pool(name="gates", bufs=2))
        psum_y = ctx.enter_context(tc.tile_pool(name="psum_y", bufs=2, space="PSUM"))
        psum_tr = ctx.enter_context(tc.tile_pool(name="psum_tr", bufs=2, space="PSUM"))
        psum_t = ctx.enter_context(tc.tile_pool(name="psum_t", bufs=2, space="PSUM"))

        wt_sb = consts.tile([D, E * D], BF16)
        nc.sync.dma_start(out=wt_sb, in_=wt[:, :])
        ident_b = consts.tile([P, P], BF16)
        nc.sync.dma_start(out=ident_b, in_=identb[:, :])
        # b4: bias rows replicated at partition offsets 0/32/64/96
        b4_sb = consts.tile([P, D], BF16)
        nc.vector.memset(b4_sb, 0.0)
        for k in range(4):
            nc.sync.dma_start(out=b4_sb[32 * k : 32 * k + E, :], in_=b_bf[:, :])
        # ramp8: [P, 8] f32 = 0..7 along free dim (for mask reconstruction)
        ramp = consts.tile([P, E], F32)
        for e in range(E):
            nc.vector.memset(ramp[:, e : e + 1], float(e))

        def body(s, base):
            xq_in = io_pool.tile([P, G, D], I8, tag="xq_in")
            nc.sync.dma_start(out=xq_in, in_=x_v[s][base])
            ga_sb = gates.tile([P, G, GROW], U8, tag="ga_sb")
            nc.sync.dma_start(out=ga_sb, in_=ga_v[s][base])
            gsv = ga_sb[:, :, 0:16].bitcast(BF16)  # [P, G, 8]
            axv = ga_sb[:, :, 16:18].bitcast(BF16)  # [P, G, 1]
            idv = ga_sb[:, :, 18:20]  # [P, G, 2] u8

            # one upcast for the whole group's x tiles
            xb_g = work.tile([P, G, D], BF16, tag="xbg")
            nc.scalar.copy(xb_g, xq_in)

            gs32 = gates.tile([P, G, E], F32, tag="gs32")
            nc.scalar.copy(gs32, gsv)
            idf = gates.tile([P, G, 2], F32, tag="idf")
            nc.scalar.copy(idf, idv)
            ax32 = gates.tile([P, G], F32, tag="ax32")
            nc.scalar.copy(ax32, axv.rearrange("p a o -> p (a o)"))

            # mask: mk[p,a,e] = (idx0==e) + (idx1==e)
            rampb = bass.AP(
                tensor=ramp.tensor, offset=ramp.offset,
                ap=[ramp.ap[0], [0, G], [1, E]],
            )
            mk = gates.tile([P, G, E], F32, tag="mk")
            id0 = bass.AP(
                tensor=idf.tensor, offset=idf.offset,
                ap=[idf.ap[0], [2, G], [0, E]],
            )
            id1 = bass.AP(
                tensor=idf.tensor, offset=idf.offset + 1,
                ap=[idf.ap[0], [2, G], [0, E]],
            )
            m1t = gates.tile([P, G, E], F32, tag="m1t")
            nc.vector.tensor_tensor(out=mk, in0=id0, in1=rampb, op=OP.is_equal)
            nc.vector.tensor_tensor(out=m1t, in0=id1, in1=rampb, op=OP.is_equal)
            nc.vector.tensor_tensor(out=mk, in0=mk, in1=m1t, op=OP.add)
            # masked expert weights with the x-quant scale folded in:
            # gmk = g * mask * axs
            gmk = gates.tile([P, G, E], F32, tag="gmk")
            nc.vector.tensor_tensor(out=gmk, in0=gs32, in1=mk, op=OP.mult)
            axb = bass.AP(
                tensor=ax32.tensor, offset=ax32.offset,
                ap=[ax32.ap[0], [1, G], [0, E]],
            )
            nc.vector.tensor_tensor(out=gmk, in0=gmk, in1=axb, op=OP.mult)

            # gT for the PE bias matmuls: pad g into 32-lane slots, transpose
            gu_pad = gates.tile([P, nh, 4, 32], BF16, tag="gu_pad")
            nc.vector.memset(gu_pad, 0.0)
            nc.vector.tensor_copy(
                out=gu_pad[:, :, :, 0:E],
                in_=gsv.rearrange("p (h q) e -> p h q e", q=4),
            )
            gt2 = gates.tile([P, nh, P], BF16, tag="gt2")
            for h in range(nh):
                gt_ps = psum_t.tile([P, D], F32, tag="tp")
                gtb = gt_ps.bitcast(BF16)[:, 0:P]
                nc.tensor.transpose(
                    gtb, gu_pad[:, h, :, :].rearrange("p q e -> p (q e)"), ident_b
                )
                nc.scalar.copy(gt2[:, h, :], gtb)

            # x transposes batched 8 per PSUM bank, one ACT copy out per half
            xts_g = xt_pool.tile([P, G, D], BF16, tag="xts")
            for half in range(G // 8):
                ptr = psum_tr.tile([P, 512], F32, tag="ptr")
                pb = ptr.bitcast(BF16)  # [P, 1024]
                for jj in range(8):
                    j = half * 8 + jj
                    nc.tensor.transpose(
                        pb[:, jj * P : (jj + 1) * P], xb_g[:, j, :], ident_b
                    )
                nc.scalar.copy(
                    xts_g[:, half * 8 : (half + 1) * 8, :],
                    pb.rearrange("p (a d) -> p a d", d=D),
                )

            s1g = work.tile([P, G, D], F32, tag="s1g")

            for j in range(G):
                yp = psum_y.tile([P, E * D], F32, tag="yp")
                nc.tensor.matmul(
                    yp[:, 0:512], xts_g[:, j, :], wt_sb[:, 0:512],
                    start=True, stop=True,
                )
                nc.tensor.matmul(
                    yp[:, 512:1024], xts_g[:, j, :], wt_sb[:, 512:1024],
                    start=True, stop=True,
                )
                h, q = j // 4, j % 4
                bp = psum_t.tile([P, D], F32, tag="tp")
                nc.tensor.matmul(
                    bp,
                    gt2[32 * q : 32 * q + E, h, :],
                    b4_sb[32 * q : 32 * q + E, :],
                    start=True, stop=True,
                    tile_position=(32 * q, 0),
                )

                # sc[p, 0:8, f] = yp * gmk bcast; sc[p, 8, f] = bp
                gmj = gmk[:, j, :]
                gmb = bass.AP(
                    tensor=gmj.tensor, offset=gmj.offset,
                    ap=[gmj.ap[0], [1, E], [0, D]],
                )
                sc = work.tile([P, E + 1, D], BF16, tag="sc")
                yp3 = yp.rearrange("p (e f) -> p e f", f=D)
                nc.vector.tensor_tensor(out=sc[:, 0:E, :], in0=yp3, in1=gmb, op=OP.mult)
                nc.scalar.copy(sc[:, E, :], bp)
                # s1[p, f] = sum over the 9 channels
                scv = bass.AP(
                    tensor=sc.tensor, offset=sc.offset,
                    ap=[sc.ap[0], [1, D], [D, E + 1]],
                )
                nc.vector.tensor_reduce(
                    out=s1g[:, j, :], in_=scv, axis=AX.X, op=OP.add
                )

            am = gates.tile([P, G], F32, tag="am")
            nc.vector.tensor_reduce(
                out=am, in_=s1g, axis=AX.X, op=OP.max, apply_absolute_value=True
            )
            oqc_sb = io_pool.tile([P, G, ROW], I8, tag="oqc_sb")
            so_view = oqc_sb[:, :, D : D + 4].bitcast(F32)
            nc.scalar.activation(
                so_view.rearrange("p a o -> p (a o)"), am, AF.Copy, scale=1.0 / QMAX
            )
            rr = gates.tile([P, G], F32, tag="rr")
            nc.vector.reciprocal(rr, so_view.rearrange("p a o -> p (a o)"))
            rrb = bass.AP(
                tensor=rr.tensor, offset=rr.offset,
                ap=[rr.ap[0], [1, G], [0, D]],
            )
            nc.vector.tensor_tensor(
                out=oqc_sb[:, :, 0:D], in0=s1g, in1=rrb, op=OP.mult
            )
            nc.sync.dma_start(out=oq_vs[s][base], in_=oqc_sb)

        for s in range(SPLITS):
            if outer == 1:
                body(s, 0)
            else:
                with tc.For_i(0, outer, 1) as it:
                    body(s, it)

    nc.compile()
    return nc


# ---------------- host side ----------------

_POOL = ThreadPoolExecutor(max_workers=24)
_CACHE_LOCK = threading.Lock()
_RUNNER_CACHE = {}
_CONST_CACHE = {}
_BUF_CACHE = {}


def _get_runner(shard_tokens):
    with _CACHE_LOCK:
        if shard_tokens in _RUNNER_CACHE:
            return _RUNNER_CACHE[shard_tokens]
    import jax
    from jax.sharding import Mesh, PartitionSpec
    from jax.experimental.shard_map import shard_map
    from concourse import bass2jax as b2j

    b2j.install_neuronx_cc_hook()
    nc = build_nc(shard_tokens)

    partition_name = nc.partition_id_tensor.name if nc.partition_id_tensor else None
    in_names, out_names, out_avals = [], [], []
    for alloc in nc.m.functions[0].allocations:
        if not isinstance(alloc, mybir.MemoryLocationSet):
            continue
        name = alloc.memorylocations[0].name
        if alloc.kind == "ExternalInput":
            if name != partition_name:
                in_names.append(name)
        elif alloc.kind == "ExternalOutput":
            out_names.append(name)
            out_avals.append(
                jax.core.ShapedArray(
                    tuple(alloc.tensor_shape), mybir.dt.np(alloc.dtype)
                )
            )
    if partition_name is not None:
        in_names.append(partition_name)
    assert nc.dbg_addr is None, "build with debug disabled"

    def _body(*args):
        operands = list(args)
        if partition_name is not None:
            operands.append(b2j.partition_id_tensor())
        outs = b2j._bass_exec_p.bind(
            *operands,
            out_avals=tuple(out_avals),
            in_names=tuple(in_names),
            out_names=tuple(out_names),
            lowering_input_output_aliases=(),
            sim_require_finite=True,
            sim_require_nnan=True,
            nc=nc,
        )
        return tuple(outs)

    mesh = Mesh(np.asarray(jax.devices()[:N_CORES]), ("core",))
    pc, pr = PartitionSpec("core"), PartitionSpec()
    # inputs in BIR allocation order: xq, gaxi, wt, identb, b_bf
    fn = jax.jit(
        shard_map(
            _body,
            mesh=mesh,
            in_specs=(pc, pc, pr, pr, pr),
            out_specs=(pc,) * SPLITS,
            check_rep=False,
        )
    )
    runner = (fn, mesh, out_names)
    with _CACHE_LOCK:
        _RUNNER_CACHE[shard_tokens] = runner
    return runner


def _get_consts(W, b, mesh):
    import jax
    from jax.sharding import NamedSharding, PartitionSpec

    key = hashlib.blake2b(W.tobytes() + b.tobytes(), digest_size=16).digest()
    with _CACHE_LOCK:
        hit = _CONST_CACHE.get(key)
    if hit is not None:
        return hit
    wt = np.ascontiguousarray(
        W.astype(np.float32).transpose(2, 0, 1).reshape(D, E * D)
    ).astype(ml_dtypes.bfloat16)
    identb = np.eye(P, dtype=ml_dtypes.bfloat16)
    b_bf = b.astype(ml_dtypes.bfloat16)
    rep = NamedSharding(mesh, PartitionSpec())
    consts = tuple(jax.device_put(a, rep) for a in (wt, identb, b_bf))
    with _CACHE_LOCK:
        _CONST_CACHE[key] = consts
    return consts


def _bufs(n):
    with _CACHE_LOCK:
        if n not in _BUF_CACHE:
            _BUF_CACHE[n] = (
                np.empty((n, D), np.int8),
                np.empty((n,), np.float32),
                np.empty((n // N_CORES, D), np.float32),  # quant scratch
                np.empty((n, GROW), np.uint8),  # packed gate rows
            )
        return _BUF_CACHE[n]


def kernel(**inputs) -> np.ndarray:
    import jax
    from jax.sharding import NamedSharding, PartitionSpec

    x = np.asarray(inputs["x"], dtype=np.float32)
    gate_W = np.asarray(inputs["gate_W"], dtype=np.float32)
    gate_b = np.asarray(inputs["gate_b"], dtype=np.float32)
    W = np.asarray(inputs["W"], dtype=np.float32)
    b = np.asarray(inputs["b"], dtype=np.float32)

    n = x.shape[0]
    shard = n // N_CORES
    fn, mesh, out_names = _get_runner(shard)
    shard_spec = NamedSharding(mesh, PartitionSpec("core"))
    wt_d, id_d, b_d = _get_consts(W, b, mesh)

    xq_all, ax_all, tmp, pk = _bufs(n)

    # --- quantize x to int8 + per-token amax ---
    for c in range(N_CORES):
        lo, hi = c * shard, (c + 1) * shard
        xs = x[lo:hi]
        np.abs(xs, out=tmp)
        ax = tmp.max(axis=1)
        np.maximum(ax, 1e-30, out=ax)
        ax_all[lo:hi] = ax
        np.multiply(xs, (127.0 / ax)[:, None], out=tmp)
        np.rint(tmp, out=tmp)
        xq_all[lo:hi] = tmp  # exact ints in [-127,127]
    # start the 64MB upload; everything below overlaps the transfer
    xq_d = jax.device_put(xq_all, shard_spec)

    # --- gate path in f32 on host (exact top-2, no flips) ---
    logits = x @ gate_W.T
    logits += gate_b
    m = logits.max(axis=1, keepdims=True)
    np.subtract(logits, m, out=logits)
    np.exp(logits, out=logits)
    ssum = logits.sum(axis=1, keepdims=True)
    g = logits
    np.divide(g, ssum, out=g)
    # top-2 via two argmax passes (ties -> lowest index, = jax.lax.top_k)
    t1 = np.argmax(g, axis=1)
    rows = np.arange(n)
    v1 = g[rows, t1].copy()
    g[rows, t1] = -1.0
    t2 = np.argmax(g, axis=1)
    g[rows, t1] = v1
    # packed gate rows: g bf16 | axs bf16 | idx u8 x2
    pk[:, 0:16] = g.astype(ml_dtypes.bfloat16).view(np.uint8)
    np.multiply(ax_all, 1.0 / 127.0, out=ax_all)  # ax_all becomes axs
    pk[:, 16:18] = ax_all.astype(ml_dtypes.bfloat16).view(np.uint8).reshape(n, 2)
    pk[:, 18] = t1
    pk[:, 19] = t2
    ga_d = jax.device_put(pk, shard_spec)

    # --- dispatch the bass kernel on 8 cores (async) ---
    outs = fn(xq_d, ga_d, wt_d, id_d, b_d)
    by_name = dict(zip(out_names, outs))

    # --- fetch 4*8 packed pieces concurrently; dequant as they land ---
    out = np.empty((n, D), np.float32)
    piece = shard // SPLITS

    def fetch(s, sh):
        core = (sh.index[0].start or 0) // piece
        lo = core * shard + s * piece
        hi = lo + piece
        arr = np.asarray(sh.data)
        sc = arr[:, D : D + 4].copy().view(np.float32)
        np.multiply(arr[:, 0:D], sc, out=out[lo:hi])
        return None

    futs = []
    for s in range(SPLITS):
        for sh in by_name[f"oqc{s}"].addressable_shards:
            futs.append(_POOL.submit(fetch, s, sh))
    for f in futs:
        f.result()
    return out


# revision 15
# speedup vs baseline: 1.0153x; 1.0153x over previous
"""MoE top-2 routing kernel for Trainium2, 8-core data-parallel.

Problem: x [524288, 128] f32; gate Linear(128->8); 8 experts Linear(128->128).
  g = softmax(x @ gate_W.T + gate_b); top-2 mask; out = sum_e (g*mask)_e * (x @ W_e.T) + g @ b

The axon tunnel moves ~35-45 MB/s aggregate (shared between directions) and
the host has a single CPU, so wall time = bytes shipped + the host work that
cannot hide under transfers. Device compute is ~0.3 s. This version:
  - x goes up as int8 with per-token scale (64MB instead of 256MB).
  - the gate path (logits/softmax/top-2) runs on the host in f32 (tiny BLAS),
    eliminating the top-2 flips low-precision gating would cause. The device
    receives one packed 20-byte row per token: g[8] bf16, axs=amax_x/127
    bf16, top-2 indices u8[2] (10MB, one upload, one DMA per group); it
    rebuilds the mask, folds the scales, and computes the bias term g @ b on
    the PE (host BLAS is ~2 GFLOP/s, so g @ b there would cost 0.9 s).
  - output returns as int8 + per-token f32 scale packed in one row of 132
    bytes, split into 4 pieces per core: 32 concurrent fetch streams, each
    piece dequantized while the others stream.
  - the jax/shard_map executable is built once and cached; outputs are not
    donated (kernel writes every element); weight/bias consts live on device
    across calls.

Device per core (65536 tokens, 4 pieces x 8 groups of 16 tiles x 128 tokens),
token index = ((piece*8 + group)*128 + partition)*16 + tile so every DMA is
one contiguous strip per partition:
  per group: one ACT upcast of all 16 int8 tiles to bf16, rebuild top-2 mask
    from indices, gmk = g*mask*axs (f32), transpose g into gT for the PE
    bias matmuls; PE transposes batched 8-at-a-time into one PSUM bank with
    a single ACT copy out.
  per tile: 2 bf16 matmuls (all 8 experts), PE bias matmul (gT slice @ b4,
    tile_position by quadrant), DVE broadcast-mult by gmk + ACT copy of the
    bias channel -> sc[9,128], DVE reduce over the 9 channels -> s1 f32.
  per group: abs-max per token, scale = amax/126.5, reciprocal, one DVE
    round-to-nearest quantize to int8, DMA out int8+scale rows.
"""

import os
import sys

if "/opt/trn_rl_repo" not in sys.path:
    sys.path.insert(0, "/opt/trn_rl_repo")

import hashlib
import threading
from concurrent.futures import ThreadPoolExecutor
from contextlib import ExitStack

import ml_dtypes
import numpy as np

import concourse.bass as bass
import concourse.tile as tile
from concourse import bacc
from concourse import mybir

F32 = mybir.dt.float32
BF16 = mybir.dt.bfloat16
I8 = mybir.dt.int8
U8 = mybir.dt.uint8
AF = mybir.ActivationFunctionType
OP = mybir.AluOpType
AX = mybir.AxisListType

N_TOKENS = 524288
D = 128
E = 8
N_CORES = 8
P = 128
G = 16  # tiles per group
SPLITS = 4  # output pieces per core
ROW = D + 4  # oq row: 128 int8 + 4 bytes f32 scale
GROW = 20  # gate row: 8 bf16 g + bf16 axs + 2 u8 idx
QMAX = 126.5  # quant headroom so rounding/reciprocal error cannot wrap int8


def build_nc(shard_tokens: int) -> bass.Bass:
    ntiles = shard_tokens // P
    assert ntiles % (G * SPLITS) == 0
    outer = ntiles // G // SPLITS  # groups per piece
    piece = shard_tokens // SPLITS
    nh = G // 4

    nc = bacc.Bacc()
    xq = nc.dram_tensor("xq", [shard_tokens, D], I8, kind="ExternalInput")
    gaxi = nc.dram_tensor("gaxi", [shard_tokens, GROW], U8, kind="ExternalInput")
    wt = nc.dram_tensor("wt", [D, E * D], BF16, kind="ExternalInput")
    identb = nc.dram_tensor("identb", [P, P], BF16, kind="ExternalInput")
    b_bf = nc.dram_tensor("b_bf", [E, D], BF16, kind="ExternalInput")
    oqcs = [
        nc.dram_tensor(f"oqc{s}", [piece, ROW], I8, kind="ExternalOutput")
        for s in range(SPLITS)
    ]

    x_v = xq.rearrange("(s n p a) d -> s n p a d", s=SPLITS, p=P, a=G)
    ga_v = gaxi.rearrange("(s n p a) c -> s n p a c", s=SPLITS, p=P, a=G)
    oq_vs = [t.rearrange("(n p a) c -> n p a c", p=P, a=G) for t in oqcs]

    with ExitStack() as ctx:
        tc = ctx.enter_context(tile.TileContext(nc))
        consts = ctx.enter_context(tc.tile_pool(name="consts", bufs=1))
        io_pool = ctx.enter_context(tc.tile_pool(name="io", bufs=2))
        xt_pool = ctx.enter_context(tc.tile_pool(name="xts", bufs=2))
        work = ctx.enter_context(tc.tile_pool(name="work", bufs=2))
        gates = ctx.enter_context(tc.tile_pool(name="gates", bufs=2))
        psum_y = ctx.enter_context(tc.tile_pool(name="psum_y", bufs=2, space="PSUM"))
        psum_tr = ctx.enter_context(tc.tile_pool(name="psum_tr", bufs=2, space="PSUM"))
        psum_t = ctx.enter_context(tc.tile_pool(name="psum_t", bufs=2, space="PSUM"))

        wt_sb = consts.tile([D, E * D], BF16)
        nc.sync.dma_start(out=wt_sb, in_=wt[:, :])
        ident_b = consts.tile([P, P], BF16)
        nc.sync.dma_start(out=ident_b, in_=identb[:, :])
        # b4: bias rows replicated at partition offsets 0/32/64/96
        b4_sb = consts.tile([P, D], BF16)
        nc.vector.memset(b4_sb, 0.0)
        for k in range(4):
            nc.sync.dma_start(out=b4_sb[32 * k : 32 * k + E, :], in_=b_bf[:, :])
        # ramp8: [P, 8] f32 = 0..7 along free dim (for mask reconstruction)
        ramp = consts.tile([P, E], F32)
        for e in range(E):
            nc.vector.memset(ramp[:, e : e + 1], float(e))

        def body(s, base):
            xq_in = io_pool.tile([P, G, D], I8, tag="xq_in")
            nc.sync.dma_start(out=xq_in, in_=x_v[s][base])
            ga_sb = gates.tile([P, G, GROW], U8, tag="ga_sb")
            nc.sync.dma_start(out=ga_sb, in_=ga_v[s][base])
            gsv = ga_sb[:, :, 0:16].bitcast(BF16)  # [P, G, 8]
            axv = ga_sb[:, :, 16:18].bitcast(BF16)  # [P, G, 1]
            idv = ga_sb[:, :, 18:20]  # [P, G, 2] u8

            # one upcast for the whole group's x tiles
            xb_g = work.tile([P, G, D], BF16, tag="xbg")
            nc.scalar.copy(xb_g, xq_in)

            gs32 = gates.tile([P, G, E], F32, tag="gs32")
            nc.scalar.copy(gs32, gsv)
            idf = gates.tile([P, G, 2], F32, tag="idf")
            nc.scalar.copy(idf, idv)
            ax32 = gates.tile([P, G], F32, tag="ax32")
            nc.scalar.copy(ax32, axv.rearrange("p a o -> p (a o)"))

            # mask: mk[p,a,e] = (idx0==e) + (idx1==e)
            rampb = bass.AP(
                tensor=ramp.tensor, offset=ramp.offset,
                ap=[ramp.ap[0], [0, G], [1, E]],
            )
            mk = gates.tile([P, G, E], F32, tag="mk")
            id0 = bass.AP(
                tensor=idf.tensor, offset=idf.offset,
                ap=[idf.ap[0], [2, G], [0, E]],
            )
            id1 = bass.AP(
                tensor=idf.tensor, offset=idf.offset + 1,
                ap=[idf.ap[0], [2, G], [0, E]],
            )
            m1t = gates.tile([P, G, E], F32, tag="m1t")
            nc.vector.tensor_tensor(out=mk, in0=id0, in1=rampb, op=OP.is_equal)
            nc.vector.tensor_tensor(out=m1t, in0=id1, in1=rampb, op=OP.is_equal)
            nc.vector.tensor_tensor(out=mk, in0=mk, in1=m1t, op=OP.add)
            # masked expert weights with the x-quant scale folded in:
            # gmk = g * mask * axs
            gmk = gates.tile([P, G, E], F32, tag="gmk")
            nc.vector.tensor_tensor(out=gmk, in0=gs32, in1=mk, op=OP.mult)
            axb = bass.AP(
                tensor=ax32.tensor, offset=ax32.offset,
                ap=[ax32.ap[0], [1, G], [0, E]],
            )
            nc.vector.tensor_tensor(out=gmk, in0=gmk, in1=axb, op=OP.mult)

            # gT for the PE bias matmuls: pad g into 32-lane slots, transpose
            gu_pad = gates.tile([P, nh, 4, 32], BF16, tag="gu_pad")
            nc.vector.memset(gu_pad, 0.0)
            nc.vector.tensor_copy(
                out=gu_pad[:, :, :, 0:E],
                in_=gsv.rearrange("p (h q) e -> p h q e", q=4),
            )
            gt2 = gates.tile([P, nh, P], BF16, tag="gt2")
            for h in range(nh):
                gt_ps = psum_t.tile([P, D], F32, tag="tp")
                gtb = gt_ps.bitcast(BF16)[:, 0:P]
                nc.tensor.transpose(
                    gtb, gu_pad[:, h, :, :].rearrange("p q e -> p (q e)"), ident_b
                )
                nc.scalar.copy(gt2[:, h, :], gtb)

            # x transposes batched 8 per PSUM bank, one ACT copy out per half
            xts_g = xt_pool.tile([P, G, D], BF16, tag="xts")
            for half in range(G // 8):
                ptr = psum_tr.tile([P, 512], F32, tag="ptr")
                pb = ptr.bitcast(BF16)  # [P, 1024]
                for jj in range(8):
                    j = half * 8 + jj
                    nc.tensor.transpose(
                        pb[:, jj * P : (jj + 1) * P], xb_g[:, j, :], ident_b
                    )
                nc.scalar.copy(
                    xts_g[:, half * 8 : (half + 1) * 8, :],
                    pb.rearrange("p (a d) -> p a d", d=D),
                )

            s1g = work.tile([P, G, D], F32, tag="s1g")

            for j in range(G):
                yp = psum_y.tile([P, E * D], F32, tag="yp")
                nc.tensor.matmul(
                    yp[:, 0:512], xts_g[:, j, :], wt_sb[:, 0:512],
                    start=True, stop=True,
                )
                nc.tensor.matmul(
                    yp[:, 512:1024], xts_g[:, j, :], wt_sb[:, 512:1024],
                    start=True, stop=True,
                )
                h, q = j // 4, j % 4
                bp = psum_t.tile([P, D], F32, tag="tp")
                nc.tensor.matmul(
                    bp,
                    gt2[32 * q : 32 * q + E, h, :],
                    b4_sb[32 * q : 32 * q + E, :],
                    start=True, stop=True,
                    tile_position=(32 * q, 0),
                )

                # sc[p, 0:8, f] = yp * gmk bcast; sc[p, 8, f] = bp
                gmj = gmk[:, j, :]
                gmb = bass.AP(
                    tensor=gmj.tensor, offset=gmj.offset,
                    ap=[gmj.ap[0], [1, E], [0, D]],
                )
                sc = work.tile([P, E + 1, D], BF16, tag="sc")
                yp3 = yp.rearrange("p (e f) -> p e f", f=D)
                nc.vector.tensor_tensor(out=sc[:, 0:E, :], in0=yp3, in1=gmb, op=OP.mult)
                nc.scalar.copy(sc[:, E, :], bp)
                # s1[p, f] = sum over the 9 channels
                scv = bass.AP(
                    tensor=sc.tensor, offset=sc.offset,
                    ap=[sc.ap[0], [1, D], [D, E + 1]],
                )
                nc.vector.tensor_reduce(
                    out=s1g[:, j, :], in_=scv, axis=AX.X, op=OP.add
                )

            am = gates.tile([P, G], F32, tag="am")
            nc.vector.tensor_reduce(
                out=am, in_=s1g, axis=AX.X, op=OP.max, apply_absolute_value=True
            )
            oqc_sb = io_pool.tile([P, G, ROW], I8, tag="oqc_sb")
            so_view = oqc_sb[:, :, D : D + 4].bitcast(F32)
            nc.scalar.activation(
                so_view.rearrange("p a o -> p (a o)"), am, AF.Copy, scale=1.0 / QMAX
            )
            rr = gates.tile([P, G], F32, tag="rr")
            nc.vector.reciprocal(rr, so_view.rearrange("p a o -> p (a o)"))
            rrb = bass.AP(
                tensor=rr.tensor, offset=rr.offset,
                ap=[rr.ap[0], [1, G], [0, D]],
            )
            nc.vector.tensor_tensor(
                out=oqc_sb[:, :, 0:D], in0=s1g, in1=rrb, op=OP.mult
            )
            nc.sync.dma_start(out=oq_vs[s][base], in_=oqc_sb)

        for s in range(SPLITS):
            if outer == 1:
                body(s, 0)
            else:
                with tc.For_i(0, outer, 1) as it:
                    body(s, it)

    nc.compile()
    return nc


# ---------------- host side ----------------

_POOL = ThreadPoolExecutor(max_workers=24)
_CACHE_LOCK = threading.Lock()
_RUNNER_CACHE = {}
_CONST_CACHE = {}
_BUF_CACHE = {}
_PRIO_SET = False


def _yield_cpu_to_transfers():
    """Nice this thread (and pool workers spawned after it) to +10.

    The axon relay's transfer threads live in this process at nice 0 and
    the host has a single CPU; numpy work running concurrently with a
    transfer steals ~20% of the wire rate. Nice is advisory and only
    matters under contention, so pre-transfer host work is unaffected.
    Must run AFTER jax backend init so the relay threads inherit nice 0.
    """
    global _PRIO_SET
    if not _PRIO_SET:
        try:
            os.setpriority(os.PRIO_PROCESS, 0, 10)
        except OSError:
            pass
        _PRIO_SET = True


def _get_runner(shard_tokens):
    with _CACHE_LOCK:
        if shard_tokens in _RUNNER_CACHE:
            return _RUNNER_CACHE[shard_tokens]
    import jax
    from jax.sharding import Mesh, PartitionSpec
    from jax.experimental.shard_map import shard_map
    from concourse import bass2jax as b2j

    b2j.install_neuronx_cc_hook()
    nc = build_nc(shard_tokens)

    partition_name = nc.partition_id_tensor.name if nc.partition_id_tensor else None
    in_names, out_names, out_avals = [], [], []
    for alloc in nc.m.functions[0].allocations:
        if not isinstance(alloc, mybir.MemoryLocationSet):
            continue
        name = alloc.memorylocations[0].name
        if alloc.kind == "ExternalInput":
            if name != partition_name:
                in_names.append(name)
        elif alloc.kind == "ExternalOutput":
            out_names.append(name)
            out_avals.append(
                jax.core.ShapedArray(
                    tuple(alloc.tensor_shape), mybir.dt.np(alloc.dtype)
                )
            )
    if partition_name is not None:
        in_names.append(partition_name)
    assert nc.dbg_addr is None, "build with debug disabled"

    def _body(*args):
        operands = list(args)
        if partition_name is not None:
            operands.append(b2j.partition_id_tensor())
        outs = b2j._bass_exec_p.bind(
            *operands,
            out_avals=tuple(out_avals),
            in_names=tuple(in_names),
            out_names=tuple(out_names),
            lowering_input_output_aliases=(),
            sim_require_finite=True,
            sim_require_nnan=True,
            nc=nc,
        )
        return tuple(outs)

    mesh = Mesh(np.asarray(jax.devices()[:N_CORES]), ("core",))
    pc, pr = PartitionSpec("core"), PartitionSpec()
    # inputs in BIR allocation order: xq, gaxi, wt, identb, b_bf
    fn = jax.jit(
        shard_map(
            _body,
            mesh=mesh,
            in_specs=(pc, pc, pr, pr, pr),
            out_specs=(pc,) * SPLITS,
            check_rep=False,
        )
    )
    runner = (fn, mesh, out_names)
    with _CACHE_LOCK:
        _RUNNER_CACHE[shard_tokens] = runner
    return runner


def _get_consts(W, b, mesh):
    import jax
    from jax.sharding import NamedSharding, PartitionSpec

    key = hashlib.blake2b(W.tobytes() + b.tobytes(), digest_size=16).digest()
    with _CACHE_LOCK:
        hit = _CONST_CACHE.get(key)
    if hit is not None:
        return hit
    wt = np.ascontiguousarray(
        W.astype(np.float32).transpose(2, 0, 1).reshape(D, E * D)
    ).astype(ml_dtypes.bfloat16)
    identb = np.eye(P, dtype=ml_dtypes.bfloat16)
    b_bf = b.astype(ml_dtypes.bfloat16)
    rep = NamedSharding(mesh, PartitionSpec())
    consts = tuple(jax.device_put(a, rep) for a in (wt, identb, b_bf))
    with _CACHE_LOCK:
        _CONST_CACHE[key] = consts
    return consts


def _bufs(n):
    with _CACHE_LOCK:
        if n not in _BUF_CACHE:
            _BUF_CACHE[n] = (
                np.empty((n, D), np.int8),
                np.empty((n,), np.float32),
                np.empty((n // N_CORES, D), np.float32),  # quant scratch
                np.empty((n, GROW), np.uint8),  # packed gate rows
            )
        return _BUF_CACHE[n]


def kernel(**inputs) -> np.ndarray:
    import jax
    from jax.sharding import NamedSharding, PartitionSpec

    x = np.asarray(inputs["x"], dtype=np.float32)
    gate_W = np.asarray(inputs["gate_W"], dtype=np.float32)
    gate_b = np.asarray(inputs["gate_b"], dtype=np.float32)
    W = np.asarray(inputs["W"], dtype=np.float32)
    b = np.asarray(inputs["b"], dtype=np.float32)

    n = x.shape[0]
    shard = n // N_CORES
    fn, mesh, out_names = _get_runner(shard)
    _yield_cpu_to_transfers()  # after jax init: relay threads keep nice 0
    shard_spec = NamedSharding(mesh, PartitionSpec("core"))
    wt_d, id_d, b_d = _get_consts(W, b, mesh)

    xq_all, ax_all, tmp, pk = _bufs(n)

    # --- quantize x to int8 + per-token amax ---
    for c in range(N_CORES):
        lo, hi = c * shard, (c + 1) * shard
        xs = x[lo:hi]
        np.abs(xs, out=tmp)
        ax = tmp.max(axis=1)
        np.maximum(ax, 1e-30, out=ax)
        ax_all[lo:hi] = ax
        np.multiply(xs, (127.0 / ax)[:, None], out=tmp)
        np.rint(tmp, out=tmp)
        xq_all[lo:hi] = tmp  # exact ints in [-127,127]
    # start the 64MB upload; everything below overlaps the transfer
    xq_d = jax.device_put(xq_all, shard_spec)

    # --- gate path in f32 on host (exact top-2, no flips) ---
    logits = x @ gate_W.T
    logits += gate_b
    m = logits.max(axis=1, keepdims=True)
    np.subtract(logits, m, out=logits)
    np.exp(logits, out=logits)
    ssum = logits.sum(axis=1, keepdims=True)
    g = logits
    np.divide(g, ssum, out=g)
    # top-2 via two argmax passes (ties -> lowest index, = jax.lax.top_k)
    t1 = np.argmax(g, axis=1)
    rows = np.arange(n)
    v1 = g[rows, t1].copy()
    g[rows, t1] = -1.0
    t2 = np.argmax(g, axis=1)
    g[rows, t1] = v1
    # packed gate rows: g bf16 | axs bf16 | idx u8 x2
    pk[:, 0:16] = g.astype(ml_dtypes.bfloat16).view(np.uint8)
    np.multiply(ax_all, 1.0 / 127.0, out=ax_all)  # ax_all becomes axs
    pk[:, 16:18] = ax_all.astype(ml_dtypes.bfloat16).view(np.uint8).reshape(n, 2)
    pk[:, 18] = t1
    pk[:, 19] = t2
    ga_d = jax.device_put(pk, shard_spec)

    # --- dispatch the bass kernel on 8 cores (async) ---
    outs = fn(xq_d, ga_d, wt_d, id_d, b_d)
    by_name = dict(zip(out_names, outs))

    # --- fetch 4*8 packed pieces concurrently; dequant as they land ---
    out = np.empty((n, D), np.float32)
    piece = shard // SPLITS

    def fetch(s, sh):
        core = (sh.index[0].start or 0) // piece
        lo = core * shard + s * piece
        hi = lo + piece
        arr = np.asarray(sh.data)
        sc = arr[:, D : D + 4].copy().view(np.float32)
        np.multiply(arr[:, 0:D], sc, out=out[lo:hi])
        return None

    futs = []
    for s in range(SPLITS):
        for sh in by_name[f"oqc{s}"].addressable_shards:
            futs.append(_POOL.submit(fetch, s, sh))
    for f in futs:
        f.result()
    return out
